# revision 2
# baseline (speedup 1.0000x reference)
"""Trainium2 Bass kernel for single-head fused-QKV attention.

Reference computation (per batch b):
    qkv = x @ W.T + b          # x:(2048,1024)  W:(3072,1024)  b:(3072,)
    q, k, v = split(qkv, 3)
    out = softmax(q @ k.T) @ v # no 1/sqrt(d) scale, single head

Sharding: 8 cores = (4 batches) x (2 query halves of 1024 tokens each).
Each core computes Q for its 1024 queries and K/V for the full 2048-token
sequence of its batch (K/V projection duplicated within the batch pair; no
collectives).  Host-side, the token axis is rotated per-core so each core's
query half occupies tokens [0,1024) — softmax(QK^T)V is invariant to a
consistent permutation of the key/value axis, so the graph stays SPMD.

All matmuls run as float32r (fp32 with 12-bit mantissa; operand products are
exact in the fp32 accumulator) at full 1 cycle/row TensorE throughput.

Per-core phases:
  1. Qt = (W_q @ x_q^T + b_q)      [e,n] layout, e on partitions
  2. Kt = (W_k @ x^T + b_k)        [e,m] layout
  3. S = Qt^T Kt per 128-query tile -> softmax -> unnormalized P spilled to
     DRAM (rowsum reciprocals kept); frees Kt/Qt SBUF
  4. V = x @ W_v.T + b_v           [m,dv] layout (bias via rank-1 matmul)
  5. P reloaded, PE-transposed to Pt, O = Pt^T V scaled by 1/rowsum
"""

import numpy as np

import concourse.bass as bass
import concourse.tile as tile
from concourse import bacc, mybir
from concourse.bass_utils import run_bass_kernel_spmd
from concourse.masks import make_identity

F32 = mybir.dt.float32
F32R = mybir.dt.float32r
AX = mybir.AxisListType
ALU = mybir.AluOpType
ACT = mybir.ActivationFunctionType

P = 128          # partitions
D = 1024         # hidden
DC = D // P      # 8 contraction chunks
NK = 2048        # keys per batch
NQ = 1024        # queries per core
NQT = NQ // P    # 8 query tiles
NMT = NK // P    # 16 key tiles
NMC = NK // 512  # 4 key chunks of 512
NVC = D // 512   # 2 dv chunks of 512

N_CORES = 8

# set by test harness to enable NTFF profiling on the SPMD run
TRACE = False
LAST_EXEC_TIME_NS = None


def _round_fp32r(a: np.ndarray) -> np.ndarray:
    """Round fp32 values to the fp32r grid (12-bit mantissa, round-half-up)."""
    bits = np.ascontiguousarray(a, dtype=np.float32).view(np.uint32)
    r = ((bits.astype(np.uint64) + 0x800) & 0xFFFFF000).astype(np.uint32)
    return r.view(np.float32).reshape(a.shape)


def _build():
    nc = bacc.Bacc("TRN2", target_bir_lowering=False, debug=False,
                   num_devices=N_CORES)

    xt_d = nc.dram_tensor("xt", [P, DC, NK], F32R, kind="ExternalInput").ap()
    wqk_d = nc.dram_tensor("wqk", [P, 16, DC, P], F32R, kind="ExternalInput").ap()
    wv_d = nc.dram_tensor("wv", [P, DC, D], F32R, kind="ExternalInput").ap()
    bqk_d = nc.dram_tensor("bqk", [P, 16], F32, kind="ExternalInput").ap()
    bv_d = nc.dram_tensor("bv", [1, D], F32R, kind="ExternalInput").ap()
    out_d = nc.dram_tensor("out", [NQ, D], F32, kind="ExternalOutput").ap()

    with tile.TileContext(nc) as tc:
        with tc.tile_pool(name="consts", bufs=1) as consts, \
             tc.tile_pool(name="stats", bufs=1) as stats, \
             tc.tile_pool(name="pdram", bufs=1, space="DRAM") as pdram, \
             tc.tile_pool(name="xt", bufs=1) as xt_pool:

            bqk_s = consts.tile([P, 16], F32)
            nc.sync.dma_start(bqk_s[:], bqk_d[:])
            bv_s = consts.tile([1, D], F32R)
            nc.sync.dma_start(bv_s[:], bv_d[:])
            ones_f = consts.tile([1, P], F32)
            nc.vector.memset(ones_f[:], 1.0)
            ones_s = consts.tile([1, P], F32R)
            nc.vector.tensor_copy(out=ones_s[:], in_=ones_f[:])
            ident_s = consts.tile([P, P], F32)
            make_identity(nc, ident_s[:])

            recip_s = stats.tile([P, NQT], F32)
            pbuf = pdram.tile([NQ, NK], F32)

            xt_s = xt_pool.tile([P, DC, NK], F32R)
            nc.sync.dma_start(xt_s[:], xt_d[:])

            with tc.tile_pool(name="qt", bufs=1) as qt_pool:
                qt_s = qt_pool.tile([P, DC, NQ], F32R)

                # ---- phase 1: Qt projection (e on partitions, n free) ----
                with tc.tile_pool(name="wq", bufs=3) as wq_pool, \
                     tc.tile_pool(name="qps", bufs=4, space="PSUM") as qps:
                    for et in range(DC):
                        wt = wq_pool.tile([P, DC, P], F32R, tag="w")
                        nc.sync.dma_start(wt[:], wqk_d[:, et])
                        for nck in range(NQ // 512):
                            ps = qps.tile([P, 512], F32, tag="ps")
                            for dc in range(DC):
                                nc.tensor.matmul(
                                    ps[:], wt[:, dc],
                                    xt_s[:, dc, nck * 512:(nck + 1) * 512],
                                    start=(dc == 0), stop=(dc == DC - 1))
                            nc.any.tensor_scalar_add(
                                qt_s[:, et, nck * 512:(nck + 1) * 512], ps[:],
                                bqk_s[:, et:et + 1])

                with tc.tile_pool(name="kt", bufs=1) as kt_pool:
                    kt_s = kt_pool.tile([P, DC, NK], F32R)

                    # ---- phase 2: Kt projection ----
                    with tc.tile_pool(name="wk", bufs=3) as wk_pool, \
                         tc.tile_pool(name="kps", bufs=4, space="PSUM") as kps:
                        for et in range(DC):
                            wt = wk_pool.tile([P, DC, P], F32R, tag="w")
                            nc.sync.dma_start(wt[:], wqk_d[:, 8 + et])
                            for mck in range(NMC):
                                ps = kps.tile([P, 512], F32, tag="ps")
                                for dc in range(DC):
                                    nc.tensor.matmul(
                                        ps[:], wt[:, dc],
                                        xt_s[:, dc, mck * 512:(mck + 1) * 512],
                                        start=(dc == 0), stop=(dc == DC - 1))
                                nc.any.tensor_scalar_add(
                                    kt_s[:, et, mck * 512:(mck + 1) * 512],
                                    ps[:], bqk_s[:, 8 + et:9 + et])

                    # ---- phase 3: S = Qt^T Kt, softmax, spill P ----
                    with tc.tile_pool(name="sps", bufs=8, space="PSUM") as sps, \
                         tc.tile_pool(name="pp", bufs=2) as pp_pool, \
                         tc.tile_pool(name="sm", bufs=2) as sm_pool:
                        for qt in range(NQT):
                            stiles = []
                            for mck in range(NMC):
                                ps = sps.tile([P, 512], F32, tag="s")
                                for ec in range(DC):
                                    nc.tensor.matmul(
                                        ps[:],
                                        qt_s[:, ec, qt * P:(qt + 1) * P],
                                        kt_s[:, ec, mck * 512:(mck + 1) * 512],
                                        start=(ec == 0), stop=(ec == DC - 1))
                                stiles.append(ps)
                            pmax = sm_pool.tile([P, NMC], F32, tag="pmax")
                            for mck in range(NMC):
                                nc.vector.tensor_reduce(
                                    pmax[:, mck:mck + 1], stiles[mck][:],
                                    axis=AX.X, op=ALU.max)
                            nmax = sm_pool.tile([P, 1], F32, tag="nmax")
                            nc.vector.tensor_reduce(
                                nmax[:], pmax[:], axis=AX.X, op=ALU.max,
                                negate=True)
                            psum4 = sm_pool.tile([P, NMC], F32, tag="psum4")
                            ptile = pp_pool.tile([P, NK], F32, tag="p")
                            for mck in range(NMC):
                                nc.scalar.activation(
                                    ptile[:, mck * 512:(mck + 1) * 512],
                                    stiles[mck][:], ACT.Exp,
                                    bias=nmax[:, 0:1], scale=1.0,
                                    accum_out=psum4[:, mck:mck + 1])
                            rsum = sm_pool.tile([P, 1], F32, tag="rsum")
                            nc.vector.tensor_reduce(
                                rsum[:], psum4[:], axis=AX.X, op=ALU.add)
                            nc.vector.reciprocal(recip_s[:, qt:qt + 1], rsum[:])
                            nc.sync.dma_start(
                                pbuf[qt * P:(qt + 1) * P, :], ptile[:])

            # ---- phase 4: V projection ([m, dv] layout) ----
            with tc.tile_pool(name="v", bufs=1) as v_pool:
                v_s = v_pool.tile([P, NMT, D], F32R)
                with tc.tile_pool(name="wv", bufs=1) as wv_pool, \
                     tc.tile_pool(name="vps", bufs=4, space="PSUM") as vps:
                    wv_s = wv_pool.tile([P, DC, D], F32R)
                    nc.sync.dma_start(wv_s[:], wv_d[:])
                    for mt in range(NMT):
                        for dvc in range(NVC):
                            ps = vps.tile([P, 512], F32, tag="ps")
                            for dc in range(DC):
                                nc.tensor.matmul(
                                    ps[:], xt_s[:, dc, mt * P:(mt + 1) * P],
                                    wv_s[:, dc, dvc * 512:(dvc + 1) * 512],
                                    start=(dc == 0), stop=False)
                            nc.tensor.matmul(
                                ps[:], ones_s[:1, :],
                                bv_s[:1, dvc * 512:(dvc + 1) * 512],
                                start=False, stop=True)
                            nc.any.tensor_copy(
                                out=v_s[:, mt, dvc * 512:(dvc + 1) * 512],
                                in_=ps[:])

                # ---- phase 5: Pt = P^T, O = Pt^T V, scale, store ----
                with tc.tile_pool(name="pin", bufs=2) as pin_pool, \
                     tc.tile_pool(name="pts", bufs=2) as pts_pool, \
                     tc.tile_pool(name="tps", bufs=2, space="PSUM") as tps, \
                     tc.tile_pool(name="ops", bufs=4, space="PSUM") as ops_pool, \
                     tc.tile_pool(name="osb", bufs=3) as osb_pool:
                    for qt in range(NQT):
                        pin_t = pin_pool.tile([P, NK], F32, tag="pin")
                        nc.sync.dma_start(pin_t[:], pbuf[qt * P:(qt + 1) * P, :])
                        pts_t = pts_pool.tile([P, NMT, P], F32R, tag="pts")
                        for mt in range(NMT):
                            tp = tps.tile([P, P], F32, tag="tp")
                            nc.tensor.transpose(
                                tp[:], pin_t[:, mt * P:(mt + 1) * P], ident_s[:])
                            nc.any.tensor_copy(out=pts_t[:, mt], in_=tp[:])
                        for dvc in range(NVC):
                            ops = ops_pool.tile([P, 512], F32, tag="o")
                            for mt in range(NMT):
                                nc.tensor.matmul(
                                    ops[:], pts_t[:, mt],
                                    v_s[:, mt, dvc * 512:(dvc + 1) * 512],
                                    start=(mt == 0), stop=(mt == NMT - 1))
                            ot = osb_pool.tile([P, 512], F32, tag="ot")
                            nc.scalar.activation(
                                ot[:], ops[:], ACT.Copy,
                                bias=0.0, scale=recip_s[:, qt:qt + 1])
                            nc.sync.dma_start(
                                out_d[qt * P:(qt + 1) * P,
                                      dvc * 512:(dvc + 1) * 512], ot[:])

    nc.compile()
    return nc


_NC_CACHE = None


def _get_nc():
    global _NC_CACHE
    if _NC_CACHE is None:
        _NC_CACHE = _build()
    return _NC_CACHE


def _prep_inputs(x, W, b):
    """Host-side shard + pack + fp32r-round. Returns in_maps for 8 cores."""
    x = np.asarray(x, dtype=np.float32)
    W = np.asarray(W, dtype=np.float32)
    b = np.asarray(b, dtype=np.float32)

    # W packs (shared across cores)
    wqk = _round_fp32r(
        np.ascontiguousarray(
            W[:2 * D].reshape(16, P, DC, P).transpose(3, 0, 2, 1)))
    wv = _round_fp32r(
        np.ascontiguousarray(W[2 * D:].reshape(D, DC, P).transpose(2, 1, 0)))
    bqk = np.ascontiguousarray(b[:2 * D].reshape(16, P).T)
    bv = _round_fp32r(b[2 * D:].reshape(1, D))

    in_maps = []
    for c in range(N_CORES):
        bi, h = divmod(c, 2)
        xb = x[bi]
        if h:
            xb = np.concatenate([xb[NQ:], xb[:NQ]], axis=0)
        # xt[p, dc, m] = xb[m, dc*128+p]
        xt = _round_fp32r(np.ascontiguousarray(
            xb.reshape(NK, DC, P).transpose(2, 1, 0)))
        in_maps.append({"xt": xt, "wqk": wqk, "wv": wv, "bqk": bqk, "bv": bv})
    return in_maps


def kernel(x, W, b):
    global LAST_EXEC_TIME_NS
    nc = _get_nc()
    in_maps = _prep_inputs(x, W, b)
    res = run_bass_kernel_spmd(nc, in_maps, core_ids=list(range(N_CORES)),
                               trace=TRACE)
    LAST_EXEC_TIME_NS = res.exec_time_ns
    out = np.empty((4, NK, D), dtype=np.float32)
    for c in range(N_CORES):
        bi, h = divmod(c, 2)
        out[bi, h * NQ:(h + 1) * NQ, :] = res.results[c]["out"]
    return out
